# revision 1
# baseline (speedup 1.0000x reference)
"""Single-head attention (B=4, S=4096, D=A=1024, fp32 I/O) on 8 TRN2 NeuronCores.

Sharding: core c handles batch b=c//2, sequence-half h=c%2 (2048 rows).
Each core projects Q, K^T and V for its own half only; core pairs exchange
K^T/V halves with chunked AllGathers (overlapped with projection compute), so
nothing is computed twice.  Attention then runs flash-style per 512-query
block against the full gathered sequence.

Device layout is transpose-free AND descriptor-friendly: the host passes
x^T and all weights pre-permuted into the SBUF partition layout, and every
DRAM staging buffer (Q^T round trip, K^T/V exchange buffers) is laid out so
each DMA moves >=1KB-per-partition contiguous runs with 128 descriptors --
the naive "(c p) s -> p c s" rearranges cost ~25us/MB in descriptor
processing and were the phase-1 bottleneck.

Q^T,K^T live as [A,S]-semantics (a on partitions), V as [S,A] (k on
partitions); scores are computed transposed ([k,q]); softmax normalization is
deferred to the output projection epilogue (exp without max subtraction is
safe here: scores are O(5)).  Matmul compute in bf16, accumulation fp32.
Softmax denominators accumulate on the vector engine (not the PE); one
cross-partition matmul per block finishes them.  k-tiles are enumerated in
gather order everywhere, which keeps scores, exp, sums and ctx consistent
without any index remapping.
"""

import numpy as np
import ml_dtypes

import concourse.bass as bass
import concourse.tile as tile
from concourse import mybir
from concourse.bass_utils import run_bass_kernel_spmd

BF = mybir.dt.bfloat16
F32 = mybir.dt.float32
AF = mybir.ActivationFunctionType

B, S, DIM, A = 4, 4096, 1024, 1024
SQ = S // 2          # rows handled per core (query rows and local K/V rows)
NC = DIM // 128      # d chunks
NA = A // 128        # a tiles
NK = S // 128        # k tiles (global)
QB = 512             # q block width
NQB = SQ // QB
SCALE = 1.0 / np.sqrt(np.float32(A))

N_CORES = 8
PAIRS = [[0, 1], [2, 3], [4, 5], [6, 7]]

LAST_RESULT = None   # BassKernelResults of the most recent run (for test.py)


def _split_multiwaits(nc):
    """This walrus build rejects instructions carrying more than one sem wait
    (and Drains carrying any); hoist extra waits into single-wait NoOps
    preceding the instruction on the same engine."""
    for f in nc.m.functions:
        for bb in f.blocks:
            new_insts = []
            for inst in bb.instructions:
                si = inst.sync_info
                if si is not None and si.on_wait:
                    keep = 0 if isinstance(inst, mybir.InstDrain) else 1
                    if len(si.on_wait) > keep:
                        waits = list(si.on_wait)
                        hoist, rest = waits[: len(waits) - keep], waits[len(waits) - keep :]
                        for w in hoist:
                            nop = mybir.InstNoOp(
                                name=nc.get_next_instruction_name(),
                                sync_info=mybir.SyncInfo(on_wait=[w], on_update=[]),
                                bass_nofuse=True,
                                engine=inst.engine,
                            )
                            nc.register_instruction(nop)
                            new_insts.append(nop)
                        si.on_wait.clear()
                        si.on_wait.extend(rest)
                new_insts.append(inst)
            bb.instructions[:] = new_insts


def _build():
    nc = bass.Bass()

    # all pre-permuted host-side into [partition, ...contiguous...] layout
    xp = nc.declare_dram_parameter("xp", [128, 4, NC, 512], BF, isOutput=False)
    WkT = nc.declare_dram_parameter("WkT", [128, NC, A], BF, isOutput=False)
    WqT = nc.declare_dram_parameter("WqT", [128, NC, A], BF, isOutput=False)
    WvT = nc.declare_dram_parameter("WvT", [128, NC, A], BF, isOutput=False)
    WoT = nc.declare_dram_parameter("WoT", [128, NA, DIM], BF, isOutput=False)
    bqc = nc.declare_dram_parameter("bqc", [128, NA], F32, isOutput=False)
    bkc = nc.declare_dram_parameter("bkc", [128, NA], F32, isOutput=False)
    bvb = nc.declare_dram_parameter("bvb", [128, A], BF, isOutput=False)
    bob = nc.declare_dram_parameter("bob", [128, DIM], BF, isOutput=False)
    out = nc.declare_dram_parameter("out", [SQ, DIM], F32, isOutput=True)

    with tile.TileContext(nc) as tc:
        with (
            tc.tile_pool(name="dram", bufs=1, space="DRAM") as dram,
            tc.tile_pool(name="singles", bufs=1) as singles,
        ):
            # Q^T staging: [p, qb, c, q'] so stores and reloads are contiguous
            QT_d = dram.tile([128, NQB, NC, QB], BF, name="QT_d")
            # K^T exchange: [p, half, am, k'] per chunk; V: [p, j, a]
            kt_in = [
                dram.tile([128, 2, NA, 512], BF, name=f"kt_in{c}", tag=f"kti{c}")
                for c in range(2)
            ]
            kt_out = [
                dram.tile([2, 128, 2, NA, 512], BF, name=f"kt_out{c}", tag=f"kto{c}")
                for c in range(2)
            ]
            v_in = [
                dram.tile([128, 8, A], BF, name=f"v_in{c}", tag=f"vi{c}")
                for c in range(2)
            ]
            v_out = [
                dram.tile([2, 128, 8, A], BF, name=f"v_out{c}", tag=f"vo{c}")
                for c in range(2)
            ]

            warm_in = dram.tile([1, 128], BF, name="warm_in")
            warm_out = dram.tile([2, 1, 128], BF, name="warm_out")

            v_sb = singles.tile([128, NK, A], BF)        # V resident, 8.4 MB
            bqc_sb = singles.tile([128, NA], F32)
            bob_sb = singles.tile([128, DIM], BF)
            ones_k = singles.tile([128, 1], F32)         # sums matmul lhsT
            ones_1 = singles.tile([1, 1], F32)           # row->partition matmul rhs

            # phase-2 streaming pools allocated BEFORE the phase-1 pools so
            # their SBUF addresses are disjoint from phase-1 tiles -> their
            # prefetch DMAs carry no WAR dependency on phase-1 compute
            ksp = tc.tile_pool(name="p2k", bufs=2)
            p2k = ksp.__enter__()
            qtp = tc.tile_pool(name="p2q", bufs=2)
            p2q = qtp.__enter__()

            qt_pre = {}   # phase-2 Q tiles prefetched during phase 1
            ks_pre = {}   # phase-2 K^T tiles prefetched during phase 1

            # ---------------- Phase 1: projections + K/V exchange ----------
            with (
                tc.tile_pool(name="p1w", bufs=1) as p1w,
                tc.tile_pool(name="p1x", bufs=1) as p1x,
                tc.tile_pool(name="p1st", bufs=1) as p1st,
                tc.tile_pool(name="p1pk", bufs=2, space="PSUM") as p1pk,
                tc.tile_pool(name="p1pv", bufs=2, space="PSUM") as p1pv,
            ):
                wk = p1w.tile([128, NC, A], BF, tag="wkq")
                wv = p1w.tile([128, NC, A], BF, tag="wv")
                # K/V biases live only in phase 1; keep them out of singles
                # so the phase-2 peak fits
                bkc_sb = p1st.tile([128, NA], F32, tag="bkc")
                bvb_sb = p1st.tile([128, A], BF, tag="bvb")
                # all of x^T stays resident through phase 1 so no PE input
                # depends on DMA while the collectives are saturating HBM
                xs_all = p1x.tile([128, 4, NC, 512], BF)

                # wake the collectives firmware immediately (the first
                # collective otherwise pays ~25us of startup latency in the
                # middle of the K/V exchange chain); staged through an
                # internal DRAM tile since collectives can't read I/O tensors
                nc.gpsimd.dma_start(out=warm_in[:], in_=xp[0:1, 0, 0, 0:128])
                nc.gpsimd.collective_compute(
                    "AllGather",
                    mybir.AluOpType.bypass,
                    replica_groups=PAIRS,
                    ins=[warm_in[:].opt()],
                    outs=[warm_out[:].opt()],
                )
                # prefetch everything up front as few large contiguous DMAs
                # (every load here is >=8KB-per-partition contiguous; small
                # pieces cost ~200ns/descriptor on the ring and starve the PE)
                nc.scalar.dma_start(out=wk[:, 0:2, :], in_=WkT[:, 0:2, :])
                nc.sync.dma_start(out=xs_all[:, 0], in_=xp[:, 0])
                nc.sync.dma_start(out=wk[:, 2:4, :], in_=WkT[:, 2:4, :])
                nc.sync.dma_start(out=xs_all[:, 1], in_=xp[:, 1])
                nc.gpsimd.dma_start(out=wk[:, 4:6, :], in_=WkT[:, 4:6, :])
                nc.gpsimd.dma_start(out=wk[:, 6:8, :], in_=WkT[:, 6:8, :])
                nc.gpsimd.dma_start(out=xs_all[:, 2], in_=xp[:, 2])
                nc.gpsimd.dma_start(out=xs_all[:, 3], in_=xp[:, 3])
                nc.scalar.dma_start(out=bkc_sb[:], in_=bkc[:])
                nc.scalar.dma_start(out=bvb_sb[:], in_=bvb[:])
                nc.scalar.dma_start(out=wv[:], in_=WvT[:])
                nc.scalar.dma_start(out=bqc_sb[:], in_=bqc[:])
                nc.scalar.dma_start(out=bob_sb[:], in_=bob[:])
                nc.vector.memset(ones_k[:], 1.0)
                nc.vector.memset(ones_1[:], 1.0)

                def kt_chunk(c):
                    # epilogues land in a wide staging tile; one contiguous
                    # 1MB store per half-chunk keeps the ring at 128
                    # descriptors/MB instead of per-128KB
                    for sbl in range(2):
                        sb = c * 2 + sbl
                        kst = p1st.tile([128, NA, 512], BF, tag=f"kst{sbl}")
                        for am in range(NA):
                            pk = p1pk.tile([128, 512], F32)
                            for dc in range(NC):
                                nc.tensor.matmul(
                                    pk[:],
                                    lhsT=wk[:, dc, am * 128 : (am + 1) * 128],
                                    rhs=xs_all[:, sb, dc, :],
                                    start=(dc == 0),
                                    stop=(dc == NC - 1),
                                )
                            nc.scalar.activation(
                                kst[:, am, :], pk[:], AF.Identity,
                                bias=bkc_sb[:, am : am + 1],
                            )
                        nc.scalar.dma_start(out=kt_in[c][:, sbl], in_=kst[:])
                    nc.gpsimd.collective_compute(
                        "AllGather",
                        mybir.AluOpType.bypass,
                        replica_groups=PAIRS,
                        ins=[kt_in[c][:].opt()],
                        outs=[kt_out[c][:].opt()],
                    )

                def v_chunk(c):
                    for sbl in range(2):
                        sb = c * 2 + sbl
                        vst = p1st.tile([128, 4, 1024], BF, tag=f"vst{sbl}")
                        for st in range(4):
                            pv = p1pv.tile([128, 1024], F32)
                            for half in range(2):
                                for dc in range(NC):
                                    nc.tensor.matmul(
                                        pv[:, half * 512 : (half + 1) * 512],
                                        lhsT=xs_all[:, sb, dc, st * 128 : (st + 1) * 128],
                                        rhs=wv[:, dc, half * 512 : (half + 1) * 512],
                                        start=(dc == 0),
                                        stop=(dc == NC - 1),
                                    )
                            nc.vector.tensor_add(vst[:, st, :], pv[:], bvb_sb[:])
                        # V stores ride the otherwise-idle gpsimd ring so the
                        # scalar ring stays clear for the Q^T store->reload
                        nc.gpsimd.dma_start(
                            out=v_in[c][:, sbl * 4 : (sbl + 1) * 4, :], in_=vst[:]
                        )
                    nc.gpsimd.collective_compute(
                        "AllGather",
                        mybir.AluOpType.bypass,
                        replica_groups=PAIRS,
                        ins=[v_in[c][:].opt()],
                        outs=[v_out[c][:].opt()],
                    )

                def v_sb_load_g(c):
                    # gathered V -> resident SBUF, k enumerated in gather
                    # order; first j-half of each hh on the gpsimd ring
                    for hh in range(2):
                        nc.gpsimd.dma_start(
                            out=v_sb[:, c * 16 + hh * 8 : c * 16 + hh * 8 + 4, :],
                            in_=v_out[c][hh, :, 0:4, :],
                        )

                def v_sb_load_s(c):
                    # second j-half on the scalar ring; emitted only after the
                    # Q-projection so its collective-completion wait cannot
                    # head-of-line-block phase-1 scalar work
                    for hh in range(2):
                        nc.scalar.dma_start(
                            out=v_sb[:, c * 16 + hh * 8 + 4 : c * 16 + hh * 8 + 8, :],
                            in_=v_out[c][hh, :, 4:8, :],
                        )

                # K chunks first: the exchange chain is firmware-serialized,
                # and phase 2 needs all of K^T ~55us into attention while V
                # isn't needed until ~80/~140us in
                kt_chunk(0)
                kt_chunk(1)
                # wq reuses wk's buffer (tag "wkq"); its DMA carries a WAR
                # dependency on the last K-proj matmul. It sits on the gpsimd
                # ring, where everything behind it (V stores, v_sb loads) is
                # gated even later, so the wait can't head-of-line-block
                wq = p1w.tile([128, NC, A], BF, tag="wkq")
                nc.gpsimd.dma_start(out=wq[:], in_=WqT[:])
                v_chunk(0)
                v_sb_load_g(0)
                v_chunk(1)
                v_sb_load_g(1)

                # prefetch the first two K^T tiles of phase 2 (sync ring;
                # gated only on the first K exchange)
                for half in range(2):
                    ks = p2k.tile([128, NC, 512], BF, name=f"ks0_00{half}", tag="ks")
                    nc.sync.dma_start(out=ks[:], in_=kt_out[0][0, :, half])
                    ks_pre[(0, 0, 0, half)] = ks

                # --- Q projection (overlaps the V exchanges) ---
                for qb in range(NQB):
                    # alternate staging buffers (odd blocks reuse the K
                    # staging, idle since the exchanges): a single buffer
                    # serializes each block's epilogues behind the previous
                    # block's store and stalls the PE at every qb boundary
                    qst = p1st.tile(
                        [128, NA, 512], BF,
                        tag="qst" if qb % 2 == 0 else "kst0",
                        name=f"qst{qb}",
                    )
                    for am in range(NA):
                        pq = p1pk.tile([128, 512], F32)
                        for dc in range(NC):
                            nc.tensor.matmul(
                                pq[:],
                                lhsT=wq[:, dc, am * 128 : (am + 1) * 128],
                                rhs=xs_all[:, qb, dc, :],
                                start=(dc == 0),
                                stop=(dc == NC - 1),
                            )
                        nc.scalar.activation(
                            qst[:, am, :], pq[:], AF.Identity,
                            bias=bqc_sb[:, am : am + 1],
                        )
                    # Q^T store + first-two-block reloads ride the sync ring,
                    # which is otherwise idle from here to the phase-2 loads;
                    # a backlogged ring here stalls the Q epilogues (single
                    # staging buffer) and with them the PE's PSUM recycling
                    nc.sync.dma_start(out=QT_d[:, qb], in_=qst[:])
                    if qb < 2:
                        qt = p2q.tile([128, NC, QB], BF, name=f"qt{qb}", tag="qt")
                        nc.sync.dma_start(out=qt[:], in_=QT_d[:, qb])
                        qt_pre[qb] = qt

                # remote V halves -> resident SBUF, scalar-ring portion.
                # Nothing phase-2-critical may queue behind these on the
                # scalar ring: they carry the V-exchange completion waits
                v_sb_load_s(0)
                v_sb_load_s(1)


            # ---------------- Phase 2: attention ----------------
            with (
                tc.tile_pool(name="p2w", bufs=1) as p2w,
                tc.tile_pool(name="p2e", bufs=2) as p2e,
                tc.tile_pool(name="p2a", bufs=2) as p2a,
                tc.tile_pool(name="p2c", bufs=1) as p2c,
                tc.tile_pool(name="p2ca", bufs=1) as p2ca,
                tc.tile_pool(name="p2s", bufs=1) as p2s,
                tc.tile_pool(name="p2r", bufs=1) as p2r,
                tc.tile_pool(name="p2o", bufs=2) as p2o,
                tc.tile_pool(name="pps", bufs=2, space="PSUM") as pps,
                tc.tile_pool(name="ppsum", bufs=1, space="PSUM") as ppsum,
                tc.tile_pool(name="ppt", bufs=1, space="PSUM") as ppt,
                tc.tile_pool(name="ppc", bufs=2, space="PSUM") as ppc,
                tc.tile_pool(name="ppo", bufs=2, space="PSUM") as ppo,
            ):
                # Wo lives in the space freed by the phase-1 pools; it is not
                # needed until the first output projection (~150us later)
                wo_sb = p2w.tile([128, NC, DIM], BF)
                nc.scalar.dma_start(out=wo_sb[:], in_=WoT[:])

                def do_scores(qb):
                    if qb in qt_pre:
                        qt = qt_pre.pop(qb)
                    else:
                        qt = p2q.tile([128, NC, QB], BF, name=f"qt{qb}", tag="qt")
                        nc.scalar.dma_start(out=qt[:], in_=QT_d[:, qb])
                    et = p2e.tile([128, NK, QB], BF, name=f"et{qb}", tag="et")
                    # per-partition partial softmax denominators accumulate on
                    # the vector engine as the exp tiles appear, so the PE
                    # only pays one cross-partition matmul per block
                    acc = p2a.tile([128, QB], F32, name=f"acc{qb}", tag="acc")
                    # scores^T + exp; k-tile groups of 4 share one KT load
                    for c in range(2):
                        for hh in range(2):
                            for half in range(2):
                                if (qb, c, hh, half) in ks_pre:
                                    ks = ks_pre.pop((qb, c, hh, half))
                                else:
                                    ks = p2k.tile([128, NC, 512], BF, name=f"ks{qb}_{c}{hh}{half}", tag="ks")
                                    nc.sync.dma_start(
                                        out=ks[:], in_=kt_out[c][hh, :, half]
                                    )
                                ebase = c * 16 + hh * 8 + half * 4
                                for kt4 in range(4):
                                    ps = pps.tile([128, QB], F32, name=f"ps{qb}_{ebase+kt4}", tag="ps")
                                    for ac in range(NC):
                                        nc.tensor.matmul(
                                            ps[:],
                                            lhsT=ks[:, ac, kt4 * 128 : (kt4 + 1) * 128],
                                            rhs=qt[:, ac, :],
                                            start=(ac == 0),
                                            stop=(ac == NC - 1),
                                        )
                                    nc.scalar.activation(
                                        et[:, ebase + kt4, :],
                                        ps[:],
                                        AF.Exp,
                                        scale=float(SCALE),
                                    )
                                    kt = ebase + kt4
                                    if kt == 0:
                                        nc.vector.tensor_copy(acc[:], et[:, 0, :])
                                    else:
                                        nc.vector.tensor_add(
                                            acc[:], acc[:], et[:, kt, :]
                                        )
                    return et, acc

                def do_sums(qb, acc):
                    # softmax denominators: single cross-partition matmul on
                    # the DVE-accumulated partials, then row->partition
                    p_row = ppsum.tile([1, QB], F32, name=f"p_row{qb}", tag="p_row")
                    nc.tensor.matmul(
                        p_row[:],
                        lhsT=ones_k[:, 0:1],
                        rhs=acc[:],
                        start=True,
                        stop=True,
                    )
                    srow = p2s.tile([1, QB], F32, name=f"srow{qb}", tag="srow")
                    nc.scalar.copy(srow[:], p_row[:])
                    recips = p2r.tile([128, 4], F32, name=f"recips{qb}", tag="recips")
                    for qi in range(4):
                        ptt = ppt.tile([128, 1], F32, name=f"ptt{qb}_{qi}", tag="ptt")
                        nc.tensor.matmul(
                            ptt[:],
                            lhsT=srow[0:1, qi * 128 : (qi + 1) * 128],
                            rhs=ones_1[0:1, 0:1],
                            start=True,
                            stop=True,
                        )
                        nc.vector.reciprocal(recips[:, qi : qi + 1], ptt[:])
                    return recips

                def do_ctxA(qb, et):
                    # first gather half of ctx^T: only needs cc2 chunk 0
                    ctA = p2ca.tile([128, NA, QB], BF, name=f"ctA{qb}", tag="ctA")
                    for at in range(NA):
                        pc = ppc.tile([128, QB], F32, name=f"pcA{qb}_{at}", tag="pc")
                        for kt in range(NK // 2):
                            nc.tensor.matmul(
                                pc[:],
                                lhsT=v_sb[:, kt, at * 128 : (at + 1) * 128],
                                rhs=et[:, kt, :],
                                start=(kt == 0),
                                stop=(kt == NK // 2 - 1),
                            )
                        nc.vector.tensor_copy(ctA[:, at, :], pc[:])
                    return ctA

                def do_ctxB(qb, et, ctA):
                    ct = p2c.tile([128, NA, QB], BF, name=f"ct{qb}", tag="ct")
                    for at in range(NA):
                        pc = ppc.tile([128, QB], F32, name=f"pcB{qb}_{at}", tag="pc")
                        for kt in range(NK // 2, NK):
                            nc.tensor.matmul(
                                pc[:],
                                lhsT=v_sb[:, kt, at * 128 : (at + 1) * 128],
                                rhs=et[:, kt, :],
                                start=(kt == NK // 2),
                                stop=(kt == NK - 1),
                            )
                        nc.vector.tensor_add(ct[:, at, :], pc[:], ctA[:, at, :])
                    return ct

                def do_out(qb, ct, recips):
                    # output projection + deferred softmax normalization + bias
                    for qi in range(4):
                        for half in range(2):
                            po = ppo.tile([128, 512], F32, name=f"po{qb}_{qi}{half}", tag="po")
                            for ac in range(NC):
                                nc.tensor.matmul(
                                    po[:],
                                    lhsT=ct[:, ac, qi * 128 : (qi + 1) * 128],
                                    rhs=wo_sb[:, ac, half * 512 : (half + 1) * 512],
                                    start=(ac == 0),
                                    stop=(ac == NC - 1),
                                )
                            ob = p2o.tile([128, 512], F32, name=f"ob{qb}_{qi}{half}", tag="ob")
                            nc.vector.tensor_scalar(
                                ob[:],
                                po[:],
                                recips[:, qi : qi + 1],
                                None,
                                op0=mybir.AluOpType.mult,
                            )
                            nc.vector.tensor_add(
                                ob[:], ob[:], bob_sb[:, half * 512 : (half + 1) * 512]
                            )
                            nc.sync.dma_start(
                                out=out[
                                    (qb * 4 + qi) * 128 : (qb * 4 + qi + 1) * 128,
                                    half * 512 : (half + 1) * 512,
                                ],
                                in_=ob[:],
                            )

                # software pipeline: the next block's scores are emitted
                # between ctxA and ctxB of the current block, so the PE has
                # independent work while the second V exchange (qb 0) or the
                # ctx/out chain of the current block is still settling.
                # do_sums sits after ctxA so its matmul never waits on the
                # tail of the DVE accumulation chain
                et0, acc0 = do_scores(0)
                cA0 = do_ctxA(0, et0)
                r0 = do_sums(0, acc0)
                et_next, acc_next = do_scores(1)
                ct0 = do_ctxB(0, et0, cA0)
                do_out(0, ct0, r0)
                for qb in range(1, NQB):
                    et, acc = et_next, acc_next
                    cA = do_ctxA(qb, et)
                    r = do_sums(qb, acc)
                    if qb + 1 < NQB:
                        et_next, acc_next = do_scores(qb + 1)
                    ct = do_ctxB(qb, et, cA)
                    do_out(qb, ct, r)
            qtp.__exit__(None, None, None)
            ksp.__exit__(None, None, None)

    _split_multiwaits(nc)
    return nc


_NC_CACHE = None


def _get_nc():
    global _NC_CACHE
    if _NC_CACHE is None:
        _NC_CACHE = _build()
    return _NC_CACHE


def kernel(x, Wq, bq, Wk, bk, Wv, bv, Wo, bo):
    global LAST_RESULT
    bf16 = ml_dtypes.bfloat16
    x = np.asarray(x, np.float32)

    def permw(w):
        # [out(=rows of W^T after .T), in] -> W^T [in, out] -> [128, in/128, out]
        wT = np.asarray(w, np.float32).T
        return np.ascontiguousarray(
            wT.reshape(NC, 128, wT.shape[1]).transpose(1, 0, 2)
        ).astype(bf16)

    WqTp = permw(Wq)
    WkTp = permw(Wk)
    WvTp = permw(Wv)
    WoTp = permw(Wo)
    bqc = np.ascontiguousarray(np.asarray(bq, np.float32).reshape(NA, 128).T)
    bkc = np.ascontiguousarray(np.asarray(bk, np.float32).reshape(NA, 128).T)
    bvb = np.ascontiguousarray(np.broadcast_to(np.asarray(bv, np.float32), (128, A))).astype(bf16)
    bob = np.ascontiguousarray(np.broadcast_to(np.asarray(bo, np.float32), (128, DIM))).astype(bf16)

    in_maps = []
    for c in range(N_CORES):
        b, h = c // 2, c % 2
        xTq = x[b, h * SQ : (h + 1) * SQ, :].T  # [DIM, SQ]
        # [dc*128+p, sb*512+s] -> [p, sb, dc, s]
        xp = np.ascontiguousarray(
            xTq.reshape(NC, 128, 4, 512).transpose(1, 2, 0, 3)
        ).astype(bf16)
        in_maps.append(
            {
                "xp": xp,
                "WqT": WqTp,
                "WkT": WkTp,
                "WvT": WvTp,
                "WoT": WoTp,
                "bqc": bqc,
                "bkc": bkc,
                "bvb": bvb,
                "bob": bob,
            }
        )

    nc = _get_nc()
    import os

    res = run_bass_kernel_spmd(
        nc,
        in_maps,
        core_ids=list(range(N_CORES)),
        trace=bool(os.environ.get("BASS_TRACE")),
    )
    LAST_RESULT = res

    out_full = np.empty((B, S, DIM), np.float32)
    for c in range(N_CORES):
        b, h = c // 2, c % 2
        out_full[b, h * SQ : (h + 1) * SQ, :] = np.asarray(
            res.results[c]["out"], dtype=np.float32
        )
    return out_full

